# revision 55
# baseline (speedup 1.0000x reference)
"""Trainium2 Bass kernel for the MAB-style dense transformer block.

Math (per batch element b, fp32):
    q = Q @ Wq.T + bq ; k = K @ Wk.T + bk ; v = K @ Wv.T + bv
    per head h (d=64): A = softmax((qh @ kh.T) / 16)
    Oh = qh + A @ vh
    O  = LN0(concat Oh) ; O = O + relu(O @ Wo.T + bo) ; out = LN1(O)

Strategy (cost-model-driven; CoreSim charges matmuls by output free size
x cycles/row, with fp8 DoubleRow at 0.5 cycles/row contracting 2x128):
  - Data-parallel over batch B=8 across 8 NeuronCores (no collectives).
  - Scores/attention path runs in fp8e4 scaled by 16 on q, k, v, weights
    (values ~N(0,16), ideal e4m3 range). The 16*16 score scale folds into
    the exp scale (1/4096); the 16 on v cancels against a 16-valued ones
    column that produces the softmax denominator inside the same matmul.
  - q/k/v projections: one DoubleRow matmul each per 128-row chunk
    (contraction 2x128 over the model dim).
  - scores: DoubleRow with d=64 split as 2x32; qT/kT are DMA-repacked to
    [32, head, half, n] after projection (SP queue, fp8, cheap).
  - A@V: output-stationary [q, d] orientation: stationary = exp chunk
    [128k, 2, 128q], moving = v [128k, 2, 65] (64 dims + denominator
    column). Accumulates 2 k-chunks per instruction at 65*0.5 cycles.
    Output lands directly in natural [q, d] layout - no transposes.
  - exp splits across ScalarE (native Exp, fp8 out, ~88 of 128 groups)
    and DVE (one-op Schraudolph approximation: int8 bitcast of a*x+b =
    fp8 exp, ~+-9%; softmax normalization and 2048-way averaging dilute
    this to ~3e-3 end-to-end). Only these two engines can read PSUM -
    the walrus verifier forbids GPSIMD/PSUM access and DMA-from-PSUM.
    Within ScalarE the two heads' groups alternate so the per-head
    scores->exp write-after-read chain hides under the other head.
  - Residual q is projected separately in fp32r (accuracy-critical).
  - fc runs in bf16 with a ones column appended to Wo.T so the matmul
    also emits row-sums of O, giving LN0's mean for free (LN0 folds into
    LN1 in the g0=1/b0=0/bo=0 case via relu scale + LN shift invariance).
  - Pool (SBUF-only ops) takes relu/residual-add/final-scale/rsqrt
    Newton iterations; DVE keeps reciprocal/bn_stats and the PSUM->SBUF
    copies; per-qs LN/fc post-work is staged across three slide slots so
    PE's in-order stream never waits on a same-slot copy.
"""

import math
import os
import sys

for _p in ("/opt/trn_rl_repo", "/root/.axon_site/_ro/trn_rl_repo"):
    if os.path.isdir(_p) and _p not in sys.path:
        sys.path.insert(0, _p)

import numpy as np
import ml_dtypes

import concourse.bass as bass
import concourse.bacc as bacc
import concourse.tile as tile
from concourse import mybir
from concourse.bass_utils import run_bass_kernel_spmd

F32 = mybir.dt.float32
FR = mybir.dt.float32r
BF16 = mybir.dt.bfloat16
FP8 = mybir.dt.float8e4
I8 = mybir.dt.int8
I32 = mybir.dt.int32
AF = mybir.ActivationFunctionType
ALU = mybir.AluOpType
PM = mybir.MatmulPerfMode

RSQRT_MAGIC = 0x5F3759DF

B = 8
N = 2048  # sequence length (per batch element)
D = 256  # model dim
H = 4  # heads
DH = D // H  # 64
P = 128
NCH = N // P  # 16 chunks of 128 along n/k
QB = 512  # query block
NQB = N // QB  # 4
QSUB = QB // P  # 4
KGRP = 2  # k-chunks per exp group (-> [128, 1024] exp ops)
NG2 = NCH // KGRP  # 8 score groups per head
SCALE8 = 1.0 / 4096.0  # 1/sqrt(D) / (16*16 fp8 scaling)
EPS = 1e-5
VW = DH + 1  # 65: per-head v columns + denominator column
ONEC = 16.0  # denominator-column value (cancels the 16x v scaling)

# Schraudolph exp for fp8e4 output: bitcast(int8(A8*s + B8)) ~ exp(s/4096)
A8C = float(8.0 / math.log(2.0) / 4096.0)
B8C = float(7 * 8 - 0.34)

# --- engine tuning knobs -------------------------------------------------
# exp group assignment: hi=0 (even head) and hi=1 (odd head) per group.
def _exp_eng(u, g, hi):
    # ACT ~84 groups (all h0 + ~2.5 h1/unit), DVE ~44. The h1 groups kept
    # on ACT preserve head-alternation so the scores WAR chain hides.
    if hi == 0:
        return "act"
    if g in (0, 3, 5):
        return "act"
    return "dve"


CFG = {
    "qkT_copy": "vector",
    "v_copy": "vector",
    "qn_copy": "vector",
    "zT_copy": "vector",
}

_prog_cache = {}


def _build(flags):
    (bq_nz, bk_nz, bv_nz, bo_nz, g0_nt, b0_nz, g1_nt, b1_nz) = flags
    ln0_fast = not (g0_nt or b0_nz or bo_nz)

    nc = bacc.Bacc()
    qt8_d = nc.declare_dram_parameter("qt8", [P, 2, N], FP8, isOutput=False)
    kt8_d = nc.declare_dram_parameter("kt8", [P, 2, N], FP8, isOutput=False)
    qtr_d = nc.declare_dram_parameter("qtr", [D, N], FR, isOutput=False)
    wq8_d = nc.declare_dram_parameter("wq8", [P, 2, D], FP8, isOutput=False)
    wk8_d = nc.declare_dram_parameter("wk8", [P, 2, D], FP8, isOutput=False)
    wv8_d = nc.declare_dram_parameter("wv8", [P, 2, D], FP8, isOutput=False)
    wqtr_d = nc.declare_dram_parameter("wqtr", [D, D], FR, isOutput=False)
    wob_d = nc.declare_dram_parameter("wob", [P, 2, D + 1], BF16, isOutput=False)
    wos_d = nc.declare_dram_parameter("wos", [D], F32, isOutput=False)
    bq2_d = nc.declare_dram_parameter("bq2", [D], F32, isOutput=False) if bq_nz else None
    bk2_d = nc.declare_dram_parameter("bk2", [D], F32, isOutput=False) if bk_nz else None
    bv16_d = nc.declare_dram_parameter("bv16", [D], F32, isOutput=False) if bv_nz else None
    bq_d = nc.declare_dram_parameter("bq", [D], F32, isOutput=False) if bq_nz else None
    bo_d = nc.declare_dram_parameter("bo", [D], F32, isOutput=False) if bo_nz else None
    g0_d = nc.declare_dram_parameter("g0", [D], F32, isOutput=False) if g0_nt else None
    b0_d = nc.declare_dram_parameter("b0", [D], F32, isOutput=False) if b0_nz else None
    g1_d = nc.declare_dram_parameter("g1", [D], F32, isOutput=False) if g1_nt else None
    b1_d = nc.declare_dram_parameter("b1", [D], F32, isOutput=False) if b1_nz else None
    out_d = nc.declare_dram_parameter("out", [N, D], F32, isOutput=True)
    ident_d = nc.inline_tensor(np.eye(P, dtype=np.float32), "ident")

    def bcast(ap_1d):
        # [D] dram vector -> AP that broadcasts along 128 partitions
        return bass.AP(tensor=ap_1d.tensor, offset=ap_1d.offset, ap=[[0, P], *ap_1d.ap])


    with tile.TileContext(nc) as tc:
        with (
            tc.tile_pool(name="consts", bufs=1) as consts,
            tc.tile_pool(name="statics", bufs=1) as statics,
        ):
            magic = consts.tile([P, QSUB], I32, tag="magic")
            nc.gpsimd.memset(magic, RSQRT_MAGIC)
            # warm the PE clock ramp during the input-DMA wait: one long
            # fp32 dummy matmul (~3.1us cold) so the first projections run
            # at full clock instead of the cold p-state
            pewarm = consts.tile([P, QB], F32, tag="pewarm")
            nc.gpsimd.memset(pewarm, 0.0)
            zrow = consts.tile([P, 1], F32, tag="zrow")
            nc.gpsimd.memset(zrow, 0.0)
            warm = consts.tile([P, 1], F32, tag="warm")
            # load the exp table set early, overlapped with input DMA
            nc.scalar.activation(warm, zrow, AF.Exp, scale=SCALE8)
            wts = {}
            for nm, dram, dt_, w in (
                ("wk8", wk8_d, FP8, D),
                ("wq8", wq8_d, FP8, D),
                ("wv8", wv8_d, FP8, D),
                ("wob", wob_d, BF16, D + 1),
            ):
                t = consts.tile([P, 2, w], dt_, tag=nm)
                nc.scalar.dma_start(out=t, in_=dram[:])
                wts[nm] = t
            wos_bc = consts.tile([P, D], F32, tag="wos_bc")
            nc.gpsimd.dma_start(out=wos_bc, in_=bcast(wos_d[:]))
            wqtrt = consts.tile([P, 2, D], FR, tag="wqtr")
            nc.gpsimd.dma_start(
                out=wqtrt, in_=wqtr_d[:].rearrange("(c p) e -> p c e", p=P)
            )
            wts["wqtr"] = wqtrt
            ident = consts.tile([P, P], F32, tag="ident")
            nc.gpsimd.dma_start(out=ident, in_=ident_d[:])
            # per-partition bias layout [128, 2] (chunk-major) for qT/kT epilogues
            bq2 = bk2 = None
            if bq_nz:
                bq2 = consts.tile([P, 2], F32, tag="bq2")
                nc.scalar.dma_start(out=bq2, in_=bq2_d[:].rearrange("(c p) -> p c", p=P))
                bq_bc = consts.tile([P, D], F32, tag="bq_bc")
                nc.scalar.dma_start(out=bq_bc, in_=bcast(bq_d[:]))
            if bk_nz:
                bk2 = consts.tile([P, 2], F32, tag="bk2")
                nc.scalar.dma_start(out=bk2, in_=bk2_d[:].rearrange("(c p) -> p c", p=P))
            if bv_nz:
                bv_bc = consts.tile([P, D], F32, tag="bv_bc")
                nc.scalar.dma_start(out=bv_bc, in_=bcast(bv16_d[:]))
            if bo_nz:
                bo_bc = consts.tile([P, D], F32, tag="bo_bc")
                nc.scalar.dma_start(out=bo_bc, in_=bcast(bo_d[:]))
            if g0_nt:
                g0_bc = consts.tile([P, D], F32, tag="g0_bc")
                nc.scalar.dma_start(out=g0_bc, in_=bcast(g0_d[:]))
            if b0_nz:
                b0_bc = consts.tile([P, D], F32, tag="b0_bc")
                nc.scalar.dma_start(out=b0_bc, in_=bcast(b0_d[:]))
            if g1_nt:
                g1_bc = consts.tile([P, D], F32, tag="g1_bc")
                nc.scalar.dma_start(out=g1_bc, in_=bcast(g1_d[:]))
            if b1_nz:
                b1_bc = consts.tile([P, D], F32, tag="b1_bc")
                nc.scalar.dma_start(out=b1_bc, in_=bcast(b1_d[:]))

            # long-lived activations
            qT = statics.tile([P, 2, N], FP8, tag="qT")  # (16q).T  [e, n]
            kT = statics.tile([P, 2, N], FP8, tag="kT")  # (16k).T  [e, n]
            # repacked for DoubleRow scores: [d%32, head, d-half, n]
            qT2 = statics.tile([32, H, 2, N], FP8, tag="qT2")
            kT2 = statics.tile([32, H, 2, N], FP8, tag="kT2")
            vp = statics.tile([P, NCH, H * VW], FP8, tag="vp")  # 16v + denom col
            qn = statics.tile([P, NCH, D], F32, tag="qn")  # q natural [n, e]
            ones_view = vp.rearrange("p n (h x) -> p n h x", h=H)[:, :, :, DH : DH + 1]
            nc.gpsimd.memset(ones_view, ONEC)

            def rsqrt_tile(pool, var_ap, tag, w=1):
                # 1/sqrt(var + EPS): fast-inverse-sqrt seed + 3 Newton steps.
                # Pool handles everything but the shift (shift is DVE-only).
                en = nc.gpsimd
                vpe = pool.tile([P, w], F32, tag=tag + "v", name=tag + "v")
                en.tensor_scalar(vpe, var_ap, EPS, None, ALU.add)
                u1 = pool.tile([P, w], I32, tag=tag + "u", name=tag + "u")
                nc.vector.tensor_scalar(
                    u1, vpe.bitcast(I32), 1, None, ALU.arith_shift_right
                )
                y = pool.tile([P, w], F32, tag=tag + "y", name=tag + "y")
                en.tensor_sub(y.bitcast(I32), magic[:, 0:w], u1)
                for it in range(3):
                    a = pool.tile([P, w], F32, tag=tag + "a", name=f"{tag}a{it}")
                    en.tensor_mul(a, y, y)
                    b = pool.tile([P, w], F32, tag=tag + "b", name=f"{tag}b{it}")
                    en.tensor_mul(b, a, vpe)
                    c = pool.tile([P, w], F32, tag=tag + "c", name=f"{tag}c{it}")
                    en.tensor_scalar(c, b, -0.5, 1.5, ALU.mult, ALU.add)
                    y2 = pool.tile([P, w], F32, tag=tag + "y", name=f"{tag}y{it}")
                    en.tensor_mul(y2, y, c)
                    y = y2
                return y

            with (
                tc.tile_pool(name="qkin", bufs=1) as qkin,
                tc.tile_pool(name="pscore", bufs=1, space="PSUM") as pscore,
                tc.tile_pool(name="pav", bufs=1, space="PSUM") as pav,
                tc.tile_pool(name="pmix", bufs=2, space="PSUM") as pmix,
                tc.tile_pool(name="expp", bufs=6) as expp,
                tc.tile_pool(name="rawp", bufs=4) as rawp,
                tc.tile_pool(name="Op", bufs=8) as Opool,
                tc.tile_pool(name="small", bufs=4) as small,
                tc.tile_pool(name="postp", bufs=4 if ln0_fast else 2) as postp,
            ):
                qt8_in = qkin.tile([P, 2, N], FP8, tag="qt8_in")
                kt8_in = qkin.tile([P, 2, N], FP8, tag="kt8_in")
                qtr_in = qkin.tile([P, 2, N], FR, tag="qtr_in")
                HN = N // 2
                # first n-block as its own small DMA so proj(0,0) starts
                # sooner; rest in halves. k before q.
                for t_in, t_d in ((kt8_in, kt8_d), (qt8_in, qt8_d)):
                    nc.sync.dma_start(
                        out=t_in[:, :, 0:QB], in_=t_d[:, :, 0:QB]
                    )
                for t_in, t_d in ((kt8_in, kt8_d), (qt8_in, qt8_d)):
                    nc.sync.dma_start(
                        out=t_in[:, :, QB:N], in_=t_d[:, :, QB:N]
                    )
                for half in range(2):
                    nc.gpsimd.dma_start(
                        out=qtr_in[:, :, half * HN : (half + 1) * HN],
                        in_=qtr_d[:].rearrange("(c p) n -> p c n", p=P)[
                            :, :, half * HN : (half + 1) * HN
                        ],
                    )

                def mixtile(name, shape=None):
                    return pmix.tile(shape or [P, QB], F32, tag="mix", name=name)

                nc.tensor.matmul(
                    mixtile("pewarm"), pewarm[:, 0:P], pewarm,
                    start=True, stop=True,
                )

                def copy_ps(name, dst, src):
                    # PSUM -> SBUF copy on the configured engine (ACT or DVE)
                    if CFG[name] == "act":
                        nc.scalar.activation(dst, src, AF.Copy)
                    else:
                        nc.vector.tensor_copy(dst, src)

                def proj_qkT_nb(j, nb):
                    # qT/kT e-chunk j, n-block nb: one DoubleRow matmul each,
                    # epilogue copy to fp8, then repack-DMA into qT2/kT2.
                    for src, wname, bias2, dstT, dst2, ceng in (
                        (kt8_in, "wk8", bk2, kT, kT2, "vector"),
                        (qt8_in, "wq8", bq2, qT, qT2, "act"),
                    ):
                        w = wts[wname]
                        ps = mixtile(f"ps_{wname}{j}{nb}")
                        nc.tensor.matmul(
                            ps,
                            w[:, :, j * P : (j + 1) * P],
                            src[:, :, nb * QB : (nb + 1) * QB],
                            start=True,
                            stop=True,
                            perf_mode=PM.DoubleRow,
                        )
                        dst = dstT[:, j, nb * QB : (nb + 1) * QB]
                        if bias2 is not None:
                            nc.vector.tensor_scalar(
                                dst, ps, bias2[:, j : j + 1], None, ALU.add
                            )
                        else:
                            if ceng == "act":
                                nc.scalar.activation(dst, ps, AF.Copy)
                            else:
                                nc.vector.tensor_copy(dst, ps)

                def repack_qk(j, nb=None):
                    # [32, head, d-half, n] repack for DoubleRow scores.
                    # nb=None: one full-row DMA per (tensor, head, half)
                    # (amortizes the 500ns descriptor floor); nb given:
                    # fine-grained startup pipelining for j=0.
                    cols = slice(None) if nb is None else slice(nb * QB, (nb + 1) * QB)
                    for dstT, dst2 in ((kT, kT2), (qT, qT2)):
                        for h in (2 * j, 2 * j + 1):
                            p0 = (h % 2) * 64
                            for half in range(2):
                                nc.sync.dma_start(
                                    out=dst2[:, h, half, cols],
                                    in_=dstT[
                                        p0 + half * 32 : p0 + half * 32 + 32, j, cols
                                    ],
                                )

                def proj_v(i):
                    psv = mixtile(f"ps_v{i}")[:, 0:D]
                    nc.tensor.matmul(
                        psv,
                        kt8_in[:, :, i * P : (i + 1) * P],
                        wts["wv8"],
                        start=True,
                        stop=True,
                        perf_mode=PM.DoubleRow,
                    )
                    vdst = vp[:, i, :].rearrange("p (h x) -> p h x", h=H)[:, :, 0:DH]
                    vsrc = psv.rearrange("p (h x) -> p h x", h=H)
                    if bv_nz:
                        bsrc = bv_bc[:].rearrange("p (h x) -> p h x", h=H)
                        nc.vector.scalar_tensor_tensor(
                            vdst, vsrc, 1.0, bsrc, ALU.bypass, ALU.add
                        )
                    else:
                        copy_ps("v_copy", vdst, vsrc)

                def proj_qn(i):
                    psq = mixtile(f"ps_q{i}")[:, 0:D]
                    for c in range(2):
                        nc.tensor.matmul(
                            psq,
                            qtr_in[:, c, i * P : (i + 1) * P],
                            wts["wqtr"][:, c, :],
                            start=(c == 0),
                            stop=(c == 1),
                        )
                    if bq_nz:
                        nc.vector.scalar_tensor_tensor(
                            qn[:, i, :], psq, 1.0, bq_bc, ALU.bypass, ALU.add
                        )
                    else:
                        copy_ps("qn_copy", qn[:, i, :], psq)

                def hp_unit(qb, hp, slides, Otiles, u, tail_thunks=None, defer_epi=False):
                    """Head pair (2hp, 2hp+1) for query block qb.

                    Per group g: 4 DoubleRow score matmuls -> exp on the
                    assigned engine -> (g-av_delay) A@V DoubleRow
                    accumulations into [q, d]-oriented PSUM. slides[g] are
                    extra trace thunks run inside the group's exp window.
                    """
                    h0, h1 = 2 * hp, 2 * hp + 1
                    av_delay = 1 if tail_thunks is not None else 3
                    # one full PSUM bank per head: a single accumulation
                    # group spans all qs slices (zero-region = 2KB bank)
                    avs = {
                        h: pav.tile([P, 512], F32, tag=f"av{h % 2}", name=f"av{qb}{h}")
                        for h in (h0, h1)
                    }
                    extiles = {h0: [None] * NG2, h1: [None] * NG2}
                    for g in range(NG2 + av_delay):
                        if g < NG2:
                            pss = {}
                            for i, h in enumerate((h0, h1)):
                                pss[h] = pscore.tile(
                                    [P, KGRP * QB], F32, tag=f"ps_s{i}",
                                    name=f"ps{qb}{h}{g}",
                                )
                            for kc in range(KGRP):
                                kchunk = g * KGRP + kc
                                for h in (h0, h1):
                                    nc.tensor.matmul(
                                        pss[h][:, kc * QB : (kc + 1) * QB],
                                        kT2[:, h, :, kchunk * P : (kchunk + 1) * P],
                                        qT2[:, h, :, qb * QB : (qb + 1) * QB],
                                        start=True,
                                        stop=True,
                                        perf_mode=PM.DoubleRow,
                                    )
                            for i, h in enumerate((h0, h1)):
                                ex = expp.tile(
                                    [P, KGRP * QB], FP8, tag="ex", name=f"ex{qb}{h}{g}"
                                )
                                e = _exp_eng(u, g, i)
                                if e == "act":
                                    nc.scalar.activation(ex, pss[h], AF.Exp, scale=SCALE8)
                                elif e == "pool":
                                    # ACT copies PSUM scores to SBUF; Pool
                                    # runs the Schraudolph exp from there.
                                    raw = rawp.tile(
                                        [P, KGRP * QB], F32, tag="raw",
                                        name=f"raw{qb}{h}{g}",
                                    )
                                    nc.scalar.activation(raw, pss[h], AF.Copy)
                                    nc.gpsimd.tensor_scalar(
                                        ex.bitcast(I8), raw, A8C, B8C,
                                        ALU.mult, ALU.add,
                                    )
                                else:
                                    nc.vector.tensor_scalar(
                                        ex.bitcast(I8), pss[h], A8C, B8C,
                                        ALU.mult, ALU.add,
                                    )
                                extiles[h][g] = ex
                            for thunk in slides[g] if g < len(slides) else ():
                                thunk()
                        gg = g - av_delay
                        if 0 <= gg < NG2:
                            for h in (h0, h1):
                                exv = extiles[h][gg].rearrange(
                                    "p (k q) -> p k q", k=KGRP
                                )
                                for qs in range(QSUB):
                                    nc.tensor.matmul(
                                        avs[h][:, qs * VW : (qs + 1) * VW],
                                        exv[:, :, qs * P : (qs + 1) * P],
                                        vp[:, 2 * gg : 2 * gg + 2, h * VW : (h + 1) * VW],
                                        start=(gg == 0 and qs == 0),
                                        stop=(gg == NG2 - 1 and qs == QSUB - 1),
                                        perf_mode=PM.DoubleRow,
                                        skip_group_check=True,
                                    )

                    rcps = {}

                    def epi_part(qs_list):
                        if not rcps:
                            for h in (h0, h1):
                                rcp = small.tile(
                                    [P, QSUB], F32, tag=f"rcp{h % 2}", name=f"rcp{qb}{h}"
                                )
                                den = avs[h][:, DH : DH + 1 + (QSUB - 1) * VW : VW]
                                nc.vector.reciprocal(rcp, den)
                                rcps[h] = rcp
                        for qs in qs_list:
                            i = qb * QSUB + qs
                            for h in (h0, h1):
                                # O = qh + (A @ V) / S
                                nc.vector.scalar_tensor_tensor(
                                    Otiles[qs][:, h * DH : (h + 1) * DH],
                                    avs[h][:, qs * VW : qs * VW + DH],
                                    rcps[h][:, qs : qs + 1],
                                    qn[:, i, h * DH : (h + 1) * DH],
                                    ALU.mult,
                                    ALU.add,
                                )

                    def epi_all():
                        epi_part(range(QSUB))

                    if defer_epi:
                        return (
                            lambda: epi_part((0, 1)),
                            lambda: epi_part((2, 3)),
                        )
                    epi_all()
                    if tail_thunks is not None:
                        for t in tail_thunks:
                            t()
                    return None

                def post_fast_stages(qb, qs, O):
                    # out = LN1(O + relu(psf - mu0 * colsum(WoT)))
                    # psf's extra ones column gives 256*mu0 for free.
                    # Split into 3 slide-stages so PE instructions never wait
                    # on copies issued in the same slot (in-order streams).
                    ctx = {}

                    def stage1():
                        ctx["OTt"] = postp.tile(
                            [P, D], BF16, tag="zT", name=f"OT{qb}{qs}"
                        )
                        for c in range(2):
                            pt2 = mixtile(f"pt2{qb}{qs}{c}")[:, 0:P].bitcast(FR)
                            nc.tensor.transpose(
                                pt2, O[:, c * P : (c + 1) * P], ident.bitcast(FR)
                            )
                            if qb == NQB - 1:
                                # tail: ACT is idle, keep DVE's drain short
                                nc.scalar.activation(
                                    ctx["OTt"][:, c * P : (c + 1) * P], pt2, AF.Copy
                                )
                            else:
                                copy_ps(
                                    "zT_copy", ctx["OTt"][:, c * P : (c + 1) * P], pt2
                                )

                    def stage2():
                        OTt = ctx["OTt"]
                        psf = mixtile(f"psf{qb}{qs}")[:, 0 : D + 1]
                        rt = postp.tile([P, D], F32, tag="rt", name=f"rt{qb}{qs}")
                        rr = postp.tile([P, D], F32, tag="rr", name=f"rr{qb}{qs}")
                        o2 = postp.tile([P, D], F32, tag="o2", name=f"o2{qb}{qs}")
                        mv1q = small.tile([P, 2], F32, tag="mv1q", name=f"mv1_{qb}_{qs}")
                        ctx["o2"] = o2
                        ctx["mv1q"] = mv1q
                        for c in range(2):
                            nc.tensor.matmul(
                                psf,
                                OTt[:, c * P : (c + 1) * P],
                                wts["wob"][:, c, :],
                                start=(c == 0),
                                stop=(c == 1),
                            )
                        # wos_bc holds -wos/256: rt = psf - mu0*wos
                        nc.vector.scalar_tensor_tensor(
                            rt, wos_bc, psf[:, D : D + 1], psf[:, 0:D], ALU.mult, ALU.add
                        )
                        nc.gpsimd.tensor_scalar(rr, rt, 0.0, None, ALU.max)
                        nc.gpsimd.tensor_add(o2, rr, O)
                        st1 = small.tile([P, 6], F32, tag="st1")
                        nc.vector.bn_stats(st1, o2)
                        nc.vector.bn_aggr(mv1q, st1)

                    def stage3():
                        o2 = ctx["o2"]
                        mv1q = ctx["mv1q"]
                        rstd1 = rsqrt_tile(small, mv1q[:, 1:2], f"r1{qs}", 1)
                        fin = postp.tile([P, D], F32, tag="fin")
                        nc.gpsimd.tensor_scalar(
                            fin, o2, mv1q[:, 0:1], rstd1, ALU.subtract, ALU.mult
                        )
                        if g1_nt:
                            f2 = postp.tile([P, D], F32, tag="f2")
                            nc.gpsimd.tensor_mul(f2, fin, g1_bc)
                            fin = f2
                        if b1_nz:
                            f3 = postp.tile([P, D], F32, tag="f3")
                            nc.gpsimd.tensor_add(f3, fin, b1_bc)
                            fin = f3
                        i = qb * QSUB + qs
                        nc.sync.dma_start(out=out_d[i * P : (i + 1) * P, :], in_=fin)

                    return stage1, stage2, stage3

                def post_general_qs(qb, qs, O):
                    # full LN0 with gains/biases, then fc + relu + residual
                    st = small.tile([P, 6], F32, tag="st0")
                    nc.vector.bn_stats(st, O)
                    mv0 = small.tile([P, 2], F32, tag="mv0", name=f"mv0_{qb}_{qs}")
                    nc.vector.bn_aggr(mv0, st)
                    rstd0 = rsqrt_tile(small, mv0[:, 1:2], f"r0{qs}", 1)
                    z = postp.tile([P, D], F32, tag="z")
                    nc.vector.tensor_scalar(
                        z, O, mv0[:, 0:1], rstd0, ALU.subtract, ALU.mult
                    )
                    if g0_nt:
                        z2 = postp.tile([P, D], F32, tag="z2")
                        nc.vector.tensor_mul(z2, z, g0_bc)
                        z = z2
                    if b0_nz:
                        z3 = postp.tile([P, D], F32, tag="z3")
                        nc.vector.tensor_add(z3, z, b0_bc)
                        z = z3
                    zTt = postp.tile([P, D], BF16, tag="zT", name=f"zT{qb}{qs}")
                    for c in range(2):
                        pt2 = mixtile(f"pt2{qb}{qs}{c}")[:, 0:P]
                        nc.tensor.transpose(pt2, z[:, c * P : (c + 1) * P], ident)
                        nc.vector.tensor_copy(zTt[:, c * P : (c + 1) * P], pt2)
                    psf = mixtile(f"psf{qb}{qs}")[:, 0 : D + 1]
                    for c in range(2):
                        nc.tensor.matmul(
                            psf,
                            zTt[:, c * P : (c + 1) * P],
                            wts["wob"][:, c, :],
                            start=(c == 0),
                            stop=(c == 1),
                        )
                    r = postp.tile([P, D], F32, tag="r")
                    if bo_nz:
                        rt = postp.tile([P, D], F32, tag="rt")
                        nc.vector.scalar_tensor_tensor(
                            rt, psf[:, 0:D], 1.0, bo_bc, ALU.bypass, ALU.add
                        )
                        nc.vector.tensor_scalar(r, rt, 0.0, None, ALU.max)
                    else:
                        nc.vector.tensor_scalar(r, psf[:, 0:D], 0.0, None, ALU.max)
                    o2 = postp.tile([P, D], F32, tag="o2")
                    nc.vector.tensor_add(o2, z, r)
                    st1 = small.tile([P, 6], F32, tag="st1")
                    nc.vector.bn_stats(st1, o2)
                    mv1q = small.tile([P, 2], F32, tag="mv1q", name=f"mv1_{qb}_{qs}")
                    nc.vector.bn_aggr(mv1q, st1)
                    rstd1 = rsqrt_tile(small, mv1q[:, 1:2], f"r1{qs}", 1)
                    fin = postp.tile([P, D], F32, tag="fin")
                    nc.vector.tensor_scalar(
                        fin, o2, mv1q[:, 0:1], rstd1, ALU.subtract, ALU.mult
                    )
                    if g1_nt:
                        f2 = postp.tile([P, D], F32, tag="f2")
                        nc.vector.tensor_mul(f2, fin, g1_bc)
                        fin = f2
                    if b1_nz:
                        f3 = postp.tile([P, D], F32, tag="f3")
                        nc.vector.tensor_add(f3, fin, b1_bc)
                        fin = f3
                    i = qb * QSUB + qs
                    nc.sync.dma_start(out=out_d[i * P : (i + 1) * P, :], in_=fin)

                def make_post_slides(qb, Otiles):
                    # [(slot, thunk), ...] — stages at least one slot apart so
                    # each stage's inputs are ready before PE reaches it
                    out = []
                    for qs in range(QSUB):
                        if ln0_fast:
                            s1, s2, s3 = post_fast_stages(qb, qs, Otiles[qs])
                            out.append((1 + qs, s1))
                            out.append((min(3 + qs, NG2 - 1), s2))
                            out.append((min(5 + qs, NG2 - 1), s3))
                        else:
                            out.append(
                                (min(1 + 2 * qs, NG2 - 1),
                                 lambda qs=qs: post_general_qs(qb, qs, Otiles[qs]))
                            )
                    return out

                # startup: j0 projections pipelined with per-nb repacks;
                # unit 0 group g only needs kchunks 2g,2g+1 (n-block g//2)
                for nb in range(NQB):
                    proj_qkT_nb(0, nb)
                    repack_qk(0, nb)
                proj_v(0)
                proj_v(1)
                post_pending = []
                epi_pending = None
                Omap = {}
                for qb in range(NQB):
                    Omap[qb] = [
                        Opool.tile([P, D], FR, tag="O", name=f"O_{qb}_{j}")
                        for j in range(QSUB)
                    ]
                    for hp in range(2):
                        u = qb * 2 + hp
                        slides = [[] for _ in range(NG2)]
                        if qb == 0 and hp == 0:
                            slides[0].append(lambda: proj_qkT_nb(1, 0))
                            slides[1].append(lambda: proj_qkT_nb(1, 1))
                            slides[2].append(lambda: proj_qkT_nb(1, 2))
                            slides[3].append(lambda: proj_qkT_nb(1, 3))
                            slides[4].append(lambda: repack_qk(1))
                            for g in range(NG2 - 1):
                                slides[g].append(lambda g=g: proj_v(2 * g + 2))
                                slides[g].append(lambda g=g: proj_v(2 * g + 3))
                            for g in range(1, 5):
                                slides[g].append(lambda g=g: proj_qn(g - 1))
                        elif qb == 0 and hp == 1:
                            for g in range(NG2):
                                slides[g].append(lambda g=g: proj_qn(4 + g))
                        elif qb == 1 and hp == 1:
                            for g in range(4):
                                slides[g].append(lambda g=g: proj_qn(12 + g))
                        if epi_pending is not None:
                            slides[0].insert(0, epi_pending[0])
                            slides[1].insert(0, epi_pending[1])
                            epi_pending = None
                        if post_pending and hp == 0:
                            for slot, thunk in post_pending:
                                slides[slot].append(thunk)
                            post_pending = []
                        last = qb == NQB - 1 and hp == 1
                        tail = None
                        if last:
                            tail = [t for _, t in make_post_slides(qb, Omap[qb])]
                        epi_pending = hp_unit(
                            qb, hp, slides, Omap[qb], u,
                            tail_thunks=tail,
                            defer_epi=not last,
                        )
                    Otiles_qb = Omap.pop(qb)
                    if qb < NQB - 1:
                        post_pending = make_post_slides(qb, Otiles_qb)

    nc.compile()
    return nc


def _get_prog(flags):
    if flags not in _prog_cache:
        _prog_cache[flags] = _build(flags)
    return _prog_cache[flags]


def _prep_inputs(Q, K, Wq, bq, Wk, bk, Wv, bv, Wo, bo, g0, b0, g1, b1):
    f32 = np.float32
    E4 = ml_dtypes.float8_e4m3
    BF = ml_dtypes.bfloat16
    Q = np.asarray(Q, f32)
    K = np.asarray(K, f32)
    flags = (
        bool(np.any(np.asarray(bq) != 0)),
        bool(np.any(np.asarray(bk) != 0)),
        bool(np.any(np.asarray(bv) != 0)),
        bool(np.any(np.asarray(bo) != 0)),
        bool(np.any(np.asarray(g0) != 1)),
        bool(np.any(np.asarray(b0) != 0)),
        bool(np.any(np.asarray(g1) != 1)),
        bool(np.any(np.asarray(b1) != 0)),
    )

    def w8(W):
        # W.T scaled by 16, fp8, laid out [p, e-chunk, e'] with e = c*128+p
        wt = (np.asarray(W, f32).T * 16.0).astype(E4)
        return np.ascontiguousarray(wt.reshape(2, P, D).transpose(1, 0, 2))

    WoT = np.asarray(Wo, f32).T
    wob = np.concatenate([WoT, np.ones((D, 1), f32)], axis=1).astype(BF)
    shared = {
        "wq8": w8(Wq),
        "wk8": w8(Wk),
        "wv8": w8(Wv),
        "wqtr": np.ascontiguousarray(np.asarray(Wq, f32).T),
        "wob": np.ascontiguousarray(wob.reshape(2, P, D + 1).transpose(1, 0, 2)),
        "wos": np.ascontiguousarray(-np.asarray(Wo, f32).sum(axis=1) / D),
    }
    opt = (
        ("bq2", 16.0 * np.asarray(bq, f32), flags[0]),
        ("bq", np.asarray(bq, f32), flags[0]),
        ("bk2", 16.0 * np.asarray(bk, f32), flags[1]),
        ("bv16", 16.0 * np.asarray(bv, f32), flags[2]),
        ("bo", np.asarray(bo, f32), flags[3]),
        ("g0", np.asarray(g0, f32), flags[4]),
        ("b0", np.asarray(b0, f32), flags[5]),
        ("g1", np.asarray(g1, f32), flags[6]),
        ("b1", np.asarray(b1, f32), flags[7]),
    )
    for nm, arr, used in opt:
        if used:
            shared[nm] = np.ascontiguousarray(arr)

    def t8(X):
        # X.T fp8 laid out [p, e-chunk, n] (weights carry the 16x scaling)
        xt = np.asarray(X, f32).T.astype(E4)
        return np.ascontiguousarray(xt.reshape(2, P, N).transpose(1, 0, 2))

    in_maps = []
    for b in range(B):
        m = dict(shared)
        m["qt8"] = t8(Q[b])
        m["kt8"] = t8(K[b])
        m["qtr"] = np.ascontiguousarray(Q[b].T)
        in_maps.append(m)
    return flags, in_maps


def run(trace=False, **inputs):
    flags, in_maps = _prep_inputs(**inputs)
    nc = _get_prog(flags)
    try:
        res = run_bass_kernel_spmd(nc, in_maps, list(range(B)), trace=trace)
    except ModuleNotFoundError:
        # NTFF profile hook unavailable in slim axon images
        res = run_bass_kernel_spmd(nc, in_maps, list(range(B)), trace=False)
    out = np.stack([res.results[b]["out"] for b in range(B)]).astype(np.float32)
    return out, res


def kernel(**inputs):
    out, _ = run(trace=False, **inputs)
    return out


# revision 60
# speedup vs baseline: 1.0135x; 1.0135x over previous
"""Trainium2 Bass kernel for the MAB-style dense transformer block.

Math (per batch element b, fp32):
    q = Q @ Wq.T + bq ; k = K @ Wk.T + bk ; v = K @ Wv.T + bv
    per head h (d=64): A = softmax((qh @ kh.T) / 16)
    Oh = qh + A @ vh
    O  = LN0(concat Oh) ; O = O + relu(O @ Wo.T + bo) ; out = LN1(O)

Strategy (cost-model-driven; CoreSim charges matmuls by output free size
x cycles/row, with fp8 DoubleRow at 0.5 cycles/row contracting 2x128):
  - Data-parallel over batch B=8 across 8 NeuronCores (no collectives).
  - Scores/attention path runs in fp8e4 scaled by 16 on q, k, v, weights
    (values ~N(0,16), ideal e4m3 range). The 16*16 score scale folds into
    the exp scale (1/4096); the 16 on v cancels against a 16-valued ones
    column that produces the softmax denominator inside the same matmul.
  - q/k/v projections: one DoubleRow matmul each per 128-row chunk
    (contraction 2x128 over the model dim).
  - scores: DoubleRow with d=64 split as 2x32; qT/kT are DMA-repacked to
    [32, head, half, n] after projection (SP queue, fp8, cheap).
  - A@V: output-stationary [q, d] orientation: stationary = exp chunk
    [128k, 2, 128q], moving = v [128k, 2, 65] (64 dims + denominator
    column). Accumulates 2 k-chunks per instruction at 65*0.5 cycles.
    Output lands directly in natural [q, d] layout - no transposes.
  - exp splits across ScalarE (native Exp, fp8 out, ~88 of 128 groups)
    and DVE (one-op Schraudolph approximation: int8 bitcast of a*x+b =
    fp8 exp, ~+-9%; softmax normalization and 2048-way averaging dilute
    this to ~3e-3 end-to-end). Only these two engines can read PSUM -
    the walrus verifier forbids GPSIMD/PSUM access and DMA-from-PSUM.
    Within ScalarE the two heads' groups alternate so the per-head
    scores->exp write-after-read chain hides under the other head.
  - Residual q is projected separately in fp32r (accuracy-critical).
  - fc runs in bf16 with a ones column appended to Wo.T so the matmul
    also emits row-sums of O, giving LN0's mean for free (LN0 folds into
    LN1 in the g0=1/b0=0/bo=0 case via relu scale + LN shift invariance).
  - Pool (SBUF-only ops) takes relu/residual-add/final-scale/rsqrt
    Newton iterations; DVE keeps reciprocal/bn_stats and the PSUM->SBUF
    copies; per-qs LN/fc post-work is staged across three slide slots so
    PE's in-order stream never waits on a same-slot copy.
"""

import math
import os
import sys

for _p in ("/opt/trn_rl_repo", "/root/.axon_site/_ro/trn_rl_repo"):
    if os.path.isdir(_p) and _p not in sys.path:
        sys.path.insert(0, _p)

import numpy as np
import ml_dtypes

import concourse.bass as bass
import concourse.bacc as bacc
import concourse.tile as tile
from concourse import mybir
from concourse.bass_utils import run_bass_kernel_spmd

F32 = mybir.dt.float32
FR = mybir.dt.float32r
BF16 = mybir.dt.bfloat16
FP8 = mybir.dt.float8e4
I8 = mybir.dt.int8
I32 = mybir.dt.int32
AF = mybir.ActivationFunctionType
ALU = mybir.AluOpType
PM = mybir.MatmulPerfMode

RSQRT_MAGIC = 0x5F3759DF

B = 8
N = 2048  # sequence length (per batch element)
D = 256  # model dim
H = 4  # heads
DH = D // H  # 64
P = 128
NCH = N // P  # 16 chunks of 128 along n/k
QB = 512  # query block
NQB = N // QB  # 4
QSUB = QB // P  # 4
KGRP = 2  # k-chunks per exp group (-> [128, 1024] exp ops)
NG2 = NCH // KGRP  # 8 score groups per head
SCALE8 = 1.0 / 4096.0  # 1/sqrt(D) / (16*16 fp8 scaling)
EPS = 1e-5
VW = DH + 1  # 65: per-head v columns + denominator column
ONEC = 16.0  # denominator-column value (cancels the 16x v scaling)

# Schraudolph exp for fp8e4 output: bitcast(int8(A8*s + B8)) ~ exp(s/4096)
A8C = float(8.0 / math.log(2.0) / 4096.0)
B8C = float(7 * 8 - 0.34)

# --- engine tuning knobs -------------------------------------------------
# exp group assignment: hi=0 (even head) and hi=1 (odd head) per group.
def _exp_eng(u, g, hi):
    # ACT ~84 groups (all h0 + ~2.5 h1/unit), DVE ~44. The h1 groups kept
    # on ACT preserve head-alternation so the scores WAR chain hides.
    if hi == 0:
        return "act"
    if g in (0, 3, 5):
        return "act"
    return "dve"


CFG = {
    "qkT_copy": "vector",
    "v_copy": "vector",
    "qn_copy": "vector",
    "zT_copy": "vector",
}

_prog_cache = {}


def _build(flags):
    (bq_nz, bk_nz, bv_nz, bo_nz, g0_nt, b0_nz, g1_nt, b1_nz) = flags
    ln0_fast = not (g0_nt or b0_nz or bo_nz)

    nc = bacc.Bacc()
    qt8_d = nc.declare_dram_parameter("qt8", [P, 2, N], FP8, isOutput=False)
    kt8_d = nc.declare_dram_parameter("kt8", [P, 2, N], FP8, isOutput=False)
    qtr_d = nc.declare_dram_parameter("qtr", [D, N], FR, isOutput=False)
    wq8_d = nc.declare_dram_parameter("wq8", [P, 2, D], FP8, isOutput=False)
    wk8_d = nc.declare_dram_parameter("wk8", [P, 2, D], FP8, isOutput=False)
    wv8_d = nc.declare_dram_parameter("wv8", [P, 2, D], FP8, isOutput=False)
    wqtr_d = nc.declare_dram_parameter("wqtr", [D, D], FR, isOutput=False)
    wob_d = nc.declare_dram_parameter("wob", [P, 2, D + 1], BF16, isOutput=False)
    wos_d = nc.declare_dram_parameter("wos", [D], F32, isOutput=False)
    bq2_d = nc.declare_dram_parameter("bq2", [D], F32, isOutput=False) if bq_nz else None
    bk2_d = nc.declare_dram_parameter("bk2", [D], F32, isOutput=False) if bk_nz else None
    bv16_d = nc.declare_dram_parameter("bv16", [D], F32, isOutput=False) if bv_nz else None
    bq_d = nc.declare_dram_parameter("bq", [D], F32, isOutput=False) if bq_nz else None
    bo_d = nc.declare_dram_parameter("bo", [D], F32, isOutput=False) if bo_nz else None
    g0_d = nc.declare_dram_parameter("g0", [D], F32, isOutput=False) if g0_nt else None
    b0_d = nc.declare_dram_parameter("b0", [D], F32, isOutput=False) if b0_nz else None
    g1_d = nc.declare_dram_parameter("g1", [D], F32, isOutput=False) if g1_nt else None
    b1_d = nc.declare_dram_parameter("b1", [D], F32, isOutput=False) if b1_nz else None
    out_d = nc.declare_dram_parameter("out", [N, D], F32, isOutput=True)
    ident_d = nc.inline_tensor(np.eye(P, dtype=np.float32), "ident")

    def bcast(ap_1d):
        # [D] dram vector -> AP that broadcasts along 128 partitions
        return bass.AP(tensor=ap_1d.tensor, offset=ap_1d.offset, ap=[[0, P], *ap_1d.ap])


    with tile.TileContext(nc) as tc:
        with (
            tc.tile_pool(name="consts", bufs=1) as consts,
            tc.tile_pool(name="statics", bufs=1) as statics,
        ):
            magic = consts.tile([P, QSUB], I32, tag="magic")
            nc.gpsimd.memset(magic, RSQRT_MAGIC)
            # warm the PE clock ramp during the input-DMA wait: one long
            # fp32 dummy matmul (~3.1us cold) so the first projections run
            # at full clock instead of the cold p-state
            pewarm = consts.tile([P, QB], F32, tag="pewarm")
            nc.gpsimd.memset(pewarm, 0.0)
            zrow = consts.tile([P, 1], F32, tag="zrow")
            nc.gpsimd.memset(zrow, 0.0)
            warm = consts.tile([P, 1], F32, tag="warm")
            # load the exp table set early, overlapped with input DMA
            nc.scalar.activation(warm, zrow, AF.Exp, scale=SCALE8)
            wts = {}
            for nm, dram, dt_, w in (
                ("wk8", wk8_d, FP8, D),
                ("wq8", wq8_d, FP8, D),
                ("wv8", wv8_d, FP8, D),
                ("wob", wob_d, BF16, D + 1),
            ):
                t = consts.tile([P, 2, w], dt_, tag=nm)
                nc.scalar.dma_start(out=t, in_=dram[:])
                wts[nm] = t
            wos_bc = consts.tile([P, D], F32, tag="wos_bc")
            nc.gpsimd.dma_start(out=wos_bc, in_=bcast(wos_d[:]))
            wqtrt = consts.tile([P, 2, D], FR, tag="wqtr")
            nc.gpsimd.dma_start(
                out=wqtrt, in_=wqtr_d[:].rearrange("(c p) e -> p c e", p=P)
            )
            wts["wqtr"] = wqtrt
            ident = consts.tile([P, P], F32, tag="ident")
            nc.gpsimd.dma_start(out=ident, in_=ident_d[:])
            # per-partition bias layout [128, 2] (chunk-major) for qT/kT epilogues
            bq2 = bk2 = None
            if bq_nz:
                bq2 = consts.tile([P, 2], F32, tag="bq2")
                nc.scalar.dma_start(out=bq2, in_=bq2_d[:].rearrange("(c p) -> p c", p=P))
                bq_bc = consts.tile([P, D], F32, tag="bq_bc")
                nc.scalar.dma_start(out=bq_bc, in_=bcast(bq_d[:]))
            if bk_nz:
                bk2 = consts.tile([P, 2], F32, tag="bk2")
                nc.scalar.dma_start(out=bk2, in_=bk2_d[:].rearrange("(c p) -> p c", p=P))
            if bv_nz:
                bv_bc = consts.tile([P, D], F32, tag="bv_bc")
                nc.scalar.dma_start(out=bv_bc, in_=bcast(bv16_d[:]))
            if bo_nz:
                bo_bc = consts.tile([P, D], F32, tag="bo_bc")
                nc.scalar.dma_start(out=bo_bc, in_=bcast(bo_d[:]))
            if g0_nt:
                g0_bc = consts.tile([P, D], F32, tag="g0_bc")
                nc.scalar.dma_start(out=g0_bc, in_=bcast(g0_d[:]))
            if b0_nz:
                b0_bc = consts.tile([P, D], F32, tag="b0_bc")
                nc.scalar.dma_start(out=b0_bc, in_=bcast(b0_d[:]))
            if g1_nt:
                g1_bc = consts.tile([P, D], F32, tag="g1_bc")
                nc.scalar.dma_start(out=g1_bc, in_=bcast(g1_d[:]))
            if b1_nz:
                b1_bc = consts.tile([P, D], F32, tag="b1_bc")
                nc.scalar.dma_start(out=b1_bc, in_=bcast(b1_d[:]))

            # long-lived activations
            qT = statics.tile([P, 2, N], FP8, tag="qT")  # (16q).T  [e, n]
            kT = statics.tile([P, 2, N], FP8, tag="kT")  # (16k).T  [e, n]
            # repacked for DoubleRow scores: [d%32, head, d-half, n]
            qT2 = statics.tile([32, H, 2, N], FP8, tag="qT2")
            kT2 = statics.tile([32, H, 2, N], FP8, tag="kT2")
            vp = statics.tile([P, NCH, H * VW], FP8, tag="vp")  # 16v + denom col
            qn = statics.tile([P, NCH, D], F32, tag="qn")  # q natural [n, e]
            ones_view = vp.rearrange("p n (h x) -> p n h x", h=H)[:, :, :, DH : DH + 1]
            nc.gpsimd.memset(ones_view, ONEC)

            def rsqrt_tile(pool, var_ap, tag, w=1):
                # 1/sqrt(var + EPS): fast-inverse-sqrt seed + 3 Newton steps.
                # Pool handles everything but the shift (shift is DVE-only).
                en = nc.gpsimd
                vpe = pool.tile([P, w], F32, tag=tag + "v", name=tag + "v")
                en.tensor_scalar(vpe, var_ap, EPS, None, ALU.add)
                u1 = pool.tile([P, w], I32, tag=tag + "u", name=tag + "u")
                nc.vector.tensor_scalar(
                    u1, vpe.bitcast(I32), 1, None, ALU.arith_shift_right
                )
                y = pool.tile([P, w], F32, tag=tag + "y", name=tag + "y")
                en.tensor_sub(y.bitcast(I32), magic[:, 0:w], u1)
                for it in range(3):
                    a = pool.tile([P, w], F32, tag=tag + "a", name=f"{tag}a{it}")
                    en.tensor_mul(a, y, y)
                    b = pool.tile([P, w], F32, tag=tag + "b", name=f"{tag}b{it}")
                    en.tensor_mul(b, a, vpe)
                    c = pool.tile([P, w], F32, tag=tag + "c", name=f"{tag}c{it}")
                    en.tensor_scalar(c, b, -0.5, 1.5, ALU.mult, ALU.add)
                    y2 = pool.tile([P, w], F32, tag=tag + "y", name=f"{tag}y{it}")
                    en.tensor_mul(y2, y, c)
                    y = y2
                return y

            with (
                tc.tile_pool(name="qkin", bufs=1) as qkin,
                tc.tile_pool(name="pscore", bufs=1, space="PSUM") as pscore,
                tc.tile_pool(name="pav", bufs=1, space="PSUM") as pav,
                tc.tile_pool(name="pmix", bufs=2, space="PSUM") as pmix,
                tc.tile_pool(name="expp", bufs=6) as expp,
                tc.tile_pool(name="rawp", bufs=4) as rawp,
                tc.tile_pool(name="Op", bufs=8) as Opool,
                tc.tile_pool(name="small", bufs=4) as small,
                tc.tile_pool(name="postp", bufs=4 if ln0_fast else 2) as postp,
            ):
                qt8_in = qkin.tile([P, 2, N], FP8, tag="qt8_in")
                kt8_in = qkin.tile([P, 2, N], FP8, tag="kt8_in")
                qtr_in = qkin.tile([P, 2, N], FR, tag="qtr_in")
                HN = N // 2
                # first n-block as its own small DMA so proj(0,0) starts
                # sooner; rest in halves. k before q.
                for t_in, t_d in ((kt8_in, kt8_d), (qt8_in, qt8_d)):
                    nc.sync.dma_start(
                        out=t_in[:, :, 0:QB], in_=t_d[:, :, 0:QB]
                    )
                for t_in, t_d in ((kt8_in, kt8_d), (qt8_in, qt8_d)):
                    nc.sync.dma_start(
                        out=t_in[:, :, QB:N], in_=t_d[:, :, QB:N]
                    )
                def load_qtr(half):
                    # deferred: frees the Pool queue for startup q-repacks
                    nc.gpsimd.dma_start(
                        out=qtr_in[:, :, half * HN : (half + 1) * HN],
                        in_=qtr_d[:].rearrange("(c p) n -> p c n", p=P)[
                            :, :, half * HN : (half + 1) * HN
                        ],
                    )

                def mixtile(name, shape=None):
                    return pmix.tile(shape or [P, QB], F32, tag="mix", name=name)

                nc.tensor.matmul(
                    mixtile("pewarm"), pewarm[:, 0:P], pewarm,
                    start=True, stop=True,
                )

                def copy_ps(name, dst, src):
                    # PSUM -> SBUF copy on the configured engine (ACT or DVE)
                    if CFG[name] == "act":
                        nc.scalar.activation(dst, src, AF.Copy)
                    else:
                        nc.vector.tensor_copy(dst, src)

                def proj_qkT_nb(j, nb):
                    # qT/kT e-chunk j, n-block nb: one DoubleRow matmul each,
                    # epilogue copy to fp8, then repack-DMA into qT2/kT2.
                    for src, wname, bias2, dstT, dst2, ceng in (
                        (kt8_in, "wk8", bk2, kT, kT2, "vector"),
                        (qt8_in, "wq8", bq2, qT, qT2, "act"),
                    ):
                        w = wts[wname]
                        ps = mixtile(f"ps_{wname}{j}{nb}")
                        nc.tensor.matmul(
                            ps,
                            w[:, :, j * P : (j + 1) * P],
                            src[:, :, nb * QB : (nb + 1) * QB],
                            start=True,
                            stop=True,
                            perf_mode=PM.DoubleRow,
                        )
                        dst = dstT[:, j, nb * QB : (nb + 1) * QB]
                        if bias2 is not None:
                            nc.vector.tensor_scalar(
                                dst, ps, bias2[:, j : j + 1], None, ALU.add
                            )
                        else:
                            if ceng == "act":
                                nc.scalar.activation(dst, ps, AF.Copy)
                            else:
                                nc.vector.tensor_copy(dst, ps)

                def repack_qk(j, nb=None):
                    # [32, head, d-half, n] repack for DoubleRow scores.
                    # nb=None: one full-row DMA per (tensor, head, half)
                    # (amortizes the 500ns descriptor floor); nb given:
                    # fine-grained startup pipelining for j=0.
                    cols = slice(None) if nb is None else slice(nb * QB, (nb + 1) * QB)
                    for dstT, dst2, qeng in ((kT, kT2, nc.sync), (qT, qT2, nc.gpsimd)):
                        eng = qeng if nb is not None else nc.sync
                        for h in (2 * j, 2 * j + 1):
                            p0 = (h % 2) * 64
                            for half in range(2):
                                eng.dma_start(
                                    out=dst2[:, h, half, cols],
                                    in_=dstT[
                                        p0 + half * 32 : p0 + half * 32 + 32, j, cols
                                    ],
                                )

                def proj_v(i):
                    psv = mixtile(f"ps_v{i}")[:, 0:D]
                    nc.tensor.matmul(
                        psv,
                        kt8_in[:, :, i * P : (i + 1) * P],
                        wts["wv8"],
                        start=True,
                        stop=True,
                        perf_mode=PM.DoubleRow,
                    )
                    vdst = vp[:, i, :].rearrange("p (h x) -> p h x", h=H)[:, :, 0:DH]
                    vsrc = psv.rearrange("p (h x) -> p h x", h=H)
                    if bv_nz:
                        bsrc = bv_bc[:].rearrange("p (h x) -> p h x", h=H)
                        nc.vector.scalar_tensor_tensor(
                            vdst, vsrc, 1.0, bsrc, ALU.bypass, ALU.add
                        )
                    else:
                        copy_ps("v_copy", vdst, vsrc)

                def proj_qn(i):
                    psq = mixtile(f"ps_q{i}")[:, 0:D]
                    for c in range(2):
                        nc.tensor.matmul(
                            psq,
                            qtr_in[:, c, i * P : (i + 1) * P],
                            wts["wqtr"][:, c, :],
                            start=(c == 0),
                            stop=(c == 1),
                        )
                    if bq_nz:
                        nc.vector.scalar_tensor_tensor(
                            qn[:, i, :], psq, 1.0, bq_bc, ALU.bypass, ALU.add
                        )
                    else:
                        copy_ps("qn_copy", qn[:, i, :], psq)

                def hp_unit(qb, hp, slides, Otiles, u, tail_thunks=None, defer_epi=False):
                    """Head pair (2hp, 2hp+1) for query block qb.

                    Per group g: 4 DoubleRow score matmuls -> exp on the
                    assigned engine -> (g-av_delay) A@V DoubleRow
                    accumulations into [q, d]-oriented PSUM. slides[g] are
                    extra trace thunks run inside the group's exp window.
                    """
                    h0, h1 = 2 * hp, 2 * hp + 1
                    av_delay = 1 if tail_thunks is not None else 3
                    # one full PSUM bank per head: a single accumulation
                    # group spans all qs slices (zero-region = 2KB bank)
                    avs = {
                        h: pav.tile([P, 512], F32, tag=f"av{h % 2}", name=f"av{qb}{h}")
                        for h in (h0, h1)
                    }
                    extiles = {h0: [None] * NG2, h1: [None] * NG2}
                    for g in range(NG2 + av_delay):
                        if g < NG2:
                            pss = {}
                            for i, h in enumerate((h0, h1)):
                                pss[h] = pscore.tile(
                                    [P, KGRP * QB], F32, tag=f"ps_s{i}",
                                    name=f"ps{qb}{h}{g}",
                                )
                            for kc in range(KGRP):
                                kchunk = g * KGRP + kc
                                for h in (h0, h1):
                                    nc.tensor.matmul(
                                        pss[h][:, kc * QB : (kc + 1) * QB],
                                        kT2[:, h, :, kchunk * P : (kchunk + 1) * P],
                                        qT2[:, h, :, qb * QB : (qb + 1) * QB],
                                        start=True,
                                        stop=True,
                                        perf_mode=PM.DoubleRow,
                                    )
                            for i, h in enumerate((h0, h1)):
                                ex = expp.tile(
                                    [P, KGRP * QB], FP8, tag="ex", name=f"ex{qb}{h}{g}"
                                )
                                e = _exp_eng(u, g, i)
                                if e == "act":
                                    nc.scalar.activation(ex, pss[h], AF.Exp, scale=SCALE8)
                                elif e == "pool":
                                    # ACT copies PSUM scores to SBUF; Pool
                                    # runs the Schraudolph exp from there.
                                    raw = rawp.tile(
                                        [P, KGRP * QB], F32, tag="raw",
                                        name=f"raw{qb}{h}{g}",
                                    )
                                    nc.scalar.activation(raw, pss[h], AF.Copy)
                                    nc.gpsimd.tensor_scalar(
                                        ex.bitcast(I8), raw, A8C, B8C,
                                        ALU.mult, ALU.add,
                                    )
                                else:
                                    nc.vector.tensor_scalar(
                                        ex.bitcast(I8), pss[h], A8C, B8C,
                                        ALU.mult, ALU.add,
                                    )
                                extiles[h][g] = ex
                            for thunk in slides[g] if g < len(slides) else ():
                                thunk()
                        gg = g - av_delay
                        if 0 <= gg < NG2:
                            for h in (h0, h1):
                                exv = extiles[h][gg].rearrange(
                                    "p (k q) -> p k q", k=KGRP
                                )
                                for qs in range(QSUB):
                                    nc.tensor.matmul(
                                        avs[h][:, qs * VW : (qs + 1) * VW],
                                        exv[:, :, qs * P : (qs + 1) * P],
                                        vp[:, 2 * gg : 2 * gg + 2, h * VW : (h + 1) * VW],
                                        start=(gg == 0 and qs == 0),
                                        stop=(gg == NG2 - 1 and qs == QSUB - 1),
                                        perf_mode=PM.DoubleRow,
                                        skip_group_check=True,
                                    )

                    rcps = {}

                    def epi_part(qs_list):
                        if not rcps:
                            for h in (h0, h1):
                                rcp = small.tile(
                                    [P, QSUB], F32, tag=f"rcp{h % 2}", name=f"rcp{qb}{h}"
                                )
                                den = avs[h][:, DH : DH + 1 + (QSUB - 1) * VW : VW]
                                nc.vector.reciprocal(rcp, den)
                                rcps[h] = rcp
                        for qs in qs_list:
                            i = qb * QSUB + qs
                            for h in (h0, h1):
                                # O = qh + (A @ V) / S
                                nc.vector.scalar_tensor_tensor(
                                    Otiles[qs][:, h * DH : (h + 1) * DH],
                                    avs[h][:, qs * VW : qs * VW + DH],
                                    rcps[h][:, qs : qs + 1],
                                    qn[:, i, h * DH : (h + 1) * DH],
                                    ALU.mult,
                                    ALU.add,
                                )

                    def epi_all():
                        epi_part(range(QSUB))

                    if defer_epi:
                        return (
                            lambda: epi_part((0, 1)),
                            lambda: epi_part((2, 3)),
                        )
                    epi_all()
                    if tail_thunks is not None:
                        for t in tail_thunks:
                            t()
                    return None

                def post_fast_stages(qb, qs, O):
                    # out = LN1(O + relu(psf - mu0 * colsum(WoT)))
                    # psf's extra ones column gives 256*mu0 for free.
                    # Split into 3 slide-stages so PE instructions never wait
                    # on copies issued in the same slot (in-order streams).
                    ctx = {}

                    def stage1():
                        ctx["OTt"] = postp.tile(
                            [P, D], BF16, tag="zT", name=f"OT{qb}{qs}"
                        )
                        for c in range(2):
                            pt2 = mixtile(f"pt2{qb}{qs}{c}")[:, 0:P].bitcast(FR)
                            nc.tensor.transpose(
                                pt2, O[:, c * P : (c + 1) * P], ident.bitcast(FR)
                            )
                            if qb == NQB - 1:
                                # tail: ACT is idle, keep DVE's drain short
                                nc.scalar.activation(
                                    ctx["OTt"][:, c * P : (c + 1) * P], pt2, AF.Copy
                                )
                            else:
                                copy_ps(
                                    "zT_copy", ctx["OTt"][:, c * P : (c + 1) * P], pt2
                                )

                    def stage2():
                        OTt = ctx["OTt"]
                        psf = mixtile(f"psf{qb}{qs}")[:, 0 : D + 1]
                        rt = postp.tile([P, D], F32, tag="rt", name=f"rt{qb}{qs}")
                        rr = postp.tile([P, D], F32, tag="rr", name=f"rr{qb}{qs}")
                        o2 = postp.tile([P, D], F32, tag="o2", name=f"o2{qb}{qs}")
                        mv1q = small.tile([P, 2], F32, tag="mv1q", name=f"mv1_{qb}_{qs}")
                        ctx["o2"] = o2
                        ctx["mv1q"] = mv1q
                        for c in range(2):
                            nc.tensor.matmul(
                                psf,
                                OTt[:, c * P : (c + 1) * P],
                                wts["wob"][:, c, :],
                                start=(c == 0),
                                stop=(c == 1),
                            )
                        # wos_bc holds -wos/256: rt = psf - mu0*wos
                        nc.vector.scalar_tensor_tensor(
                            rt, wos_bc, psf[:, D : D + 1], psf[:, 0:D], ALU.mult, ALU.add
                        )
                        nc.gpsimd.tensor_scalar(rr, rt, 0.0, None, ALU.max)
                        nc.gpsimd.tensor_add(o2, rr, O)
                        st1 = small.tile([P, 6], F32, tag="st1")
                        nc.vector.bn_stats(st1, o2)
                        nc.vector.bn_aggr(mv1q, st1)

                    def stage3():
                        o2 = ctx["o2"]
                        mv1q = ctx["mv1q"]
                        rstd1 = rsqrt_tile(small, mv1q[:, 1:2], f"r1{qs}", 1)
                        fin = postp.tile([P, D], F32, tag="fin")
                        nc.gpsimd.tensor_scalar(
                            fin, o2, mv1q[:, 0:1], rstd1, ALU.subtract, ALU.mult
                        )
                        if g1_nt:
                            f2 = postp.tile([P, D], F32, tag="f2")
                            nc.gpsimd.tensor_mul(f2, fin, g1_bc)
                            fin = f2
                        if b1_nz:
                            f3 = postp.tile([P, D], F32, tag="f3")
                            nc.gpsimd.tensor_add(f3, fin, b1_bc)
                            fin = f3
                        i = qb * QSUB + qs
                        nc.sync.dma_start(out=out_d[i * P : (i + 1) * P, :], in_=fin)

                    return stage1, stage2, stage3

                def post_general_qs(qb, qs, O):
                    # full LN0 with gains/biases, then fc + relu + residual
                    st = small.tile([P, 6], F32, tag="st0")
                    nc.vector.bn_stats(st, O)
                    mv0 = small.tile([P, 2], F32, tag="mv0", name=f"mv0_{qb}_{qs}")
                    nc.vector.bn_aggr(mv0, st)
                    rstd0 = rsqrt_tile(small, mv0[:, 1:2], f"r0{qs}", 1)
                    z = postp.tile([P, D], F32, tag="z")
                    nc.vector.tensor_scalar(
                        z, O, mv0[:, 0:1], rstd0, ALU.subtract, ALU.mult
                    )
                    if g0_nt:
                        z2 = postp.tile([P, D], F32, tag="z2")
                        nc.vector.tensor_mul(z2, z, g0_bc)
                        z = z2
                    if b0_nz:
                        z3 = postp.tile([P, D], F32, tag="z3")
                        nc.vector.tensor_add(z3, z, b0_bc)
                        z = z3
                    zTt = postp.tile([P, D], BF16, tag="zT", name=f"zT{qb}{qs}")
                    for c in range(2):
                        pt2 = mixtile(f"pt2{qb}{qs}{c}")[:, 0:P]
                        nc.tensor.transpose(pt2, z[:, c * P : (c + 1) * P], ident)
                        nc.vector.tensor_copy(zTt[:, c * P : (c + 1) * P], pt2)
                    psf = mixtile(f"psf{qb}{qs}")[:, 0 : D + 1]
                    for c in range(2):
                        nc.tensor.matmul(
                            psf,
                            zTt[:, c * P : (c + 1) * P],
                            wts["wob"][:, c, :],
                            start=(c == 0),
                            stop=(c == 1),
                        )
                    r = postp.tile([P, D], F32, tag="r")
                    if bo_nz:
                        rt = postp.tile([P, D], F32, tag="rt")
                        nc.vector.scalar_tensor_tensor(
                            rt, psf[:, 0:D], 1.0, bo_bc, ALU.bypass, ALU.add
                        )
                        nc.vector.tensor_scalar(r, rt, 0.0, None, ALU.max)
                    else:
                        nc.vector.tensor_scalar(r, psf[:, 0:D], 0.0, None, ALU.max)
                    o2 = postp.tile([P, D], F32, tag="o2")
                    nc.vector.tensor_add(o2, z, r)
                    st1 = small.tile([P, 6], F32, tag="st1")
                    nc.vector.bn_stats(st1, o2)
                    mv1q = small.tile([P, 2], F32, tag="mv1q", name=f"mv1_{qb}_{qs}")
                    nc.vector.bn_aggr(mv1q, st1)
                    rstd1 = rsqrt_tile(small, mv1q[:, 1:2], f"r1{qs}", 1)
                    fin = postp.tile([P, D], F32, tag="fin")
                    nc.vector.tensor_scalar(
                        fin, o2, mv1q[:, 0:1], rstd1, ALU.subtract, ALU.mult
                    )
                    if g1_nt:
                        f2 = postp.tile([P, D], F32, tag="f2")
                        nc.vector.tensor_mul(f2, fin, g1_bc)
                        fin = f2
                    if b1_nz:
                        f3 = postp.tile([P, D], F32, tag="f3")
                        nc.vector.tensor_add(f3, fin, b1_bc)
                        fin = f3
                    i = qb * QSUB + qs
                    nc.sync.dma_start(out=out_d[i * P : (i + 1) * P, :], in_=fin)

                def make_post_slides(qb, Otiles):
                    # [(slot, thunk), ...] — stages at least one slot apart so
                    # each stage's inputs are ready before PE reaches it
                    out = []
                    for qs in range(QSUB):
                        if ln0_fast:
                            s1, s2, s3 = post_fast_stages(qb, qs, Otiles[qs])
                            out.append((1 + qs, s1))
                            out.append((min(3 + qs, NG2 - 1), s2))
                            out.append((min(5 + qs, NG2 - 1), s3))
                        else:
                            out.append(
                                (min(1 + 2 * qs, NG2 - 1),
                                 lambda qs=qs: post_general_qs(qb, qs, Otiles[qs]))
                            )
                    return out

                # startup: j0 projections pipelined with per-nb repacks;
                # unit 0 group g only needs kchunks 2g,2g+1 (n-block g//2)
                for nb in range(NQB):
                    proj_qkT_nb(0, nb)
                    repack_qk(0, nb)
                proj_v(0)
                proj_v(1)
                post_pending = []
                epi_pending = None
                Omap = {}
                for qb in range(NQB):
                    Omap[qb] = [
                        Opool.tile([P, D], FR, tag="O", name=f"O_{qb}_{j}")
                        for j in range(QSUB)
                    ]
                    for hp in range(2):
                        u = qb * 2 + hp
                        slides = [[] for _ in range(NG2)]
                        if qb == 0 and hp == 0:
                            slides[0].append(lambda: load_qtr(0))
                            slides[1].append(lambda: load_qtr(1))
                            slides[0].append(lambda: proj_qkT_nb(1, 0))
                            slides[1].append(lambda: proj_qkT_nb(1, 1))
                            slides[2].append(lambda: proj_qkT_nb(1, 2))
                            slides[3].append(lambda: proj_qkT_nb(1, 3))
                            slides[4].append(lambda: repack_qk(1))
                            for g in range(NG2 - 1):
                                slides[g].append(lambda g=g: proj_v(2 * g + 2))
                                slides[g].append(lambda g=g: proj_v(2 * g + 3))
                            for g in range(4, 8):
                                slides[g].append(lambda g=g: proj_qn(g - 4))
                        elif qb == 0 and hp == 1:
                            for g in range(NG2):
                                slides[g].append(lambda g=g: proj_qn(4 + g))
                        elif qb == 1 and hp == 1:
                            for g in range(4):
                                slides[g].append(lambda g=g: proj_qn(12 + g))
                        if epi_pending is not None:
                            slides[0].insert(0, epi_pending[0])
                            slides[1].insert(0, epi_pending[1])
                            epi_pending = None
                        if post_pending and hp == 0:
                            for slot, thunk in post_pending:
                                slides[slot].append(thunk)
                            post_pending = []
                        last = qb == NQB - 1 and hp == 1
                        tail = None
                        if last:
                            tail = [t for _, t in make_post_slides(qb, Omap[qb])]
                        epi_pending = hp_unit(
                            qb, hp, slides, Omap[qb], u,
                            tail_thunks=tail,
                            defer_epi=not last,
                        )
                    Otiles_qb = Omap.pop(qb)
                    if qb < NQB - 1:
                        post_pending = make_post_slides(qb, Otiles_qb)

    nc.compile()
    return nc


def _get_prog(flags):
    if flags not in _prog_cache:
        _prog_cache[flags] = _build(flags)
    return _prog_cache[flags]


def _prep_inputs(Q, K, Wq, bq, Wk, bk, Wv, bv, Wo, bo, g0, b0, g1, b1):
    f32 = np.float32
    E4 = ml_dtypes.float8_e4m3
    BF = ml_dtypes.bfloat16
    Q = np.asarray(Q, f32)
    K = np.asarray(K, f32)
    flags = (
        bool(np.any(np.asarray(bq) != 0)),
        bool(np.any(np.asarray(bk) != 0)),
        bool(np.any(np.asarray(bv) != 0)),
        bool(np.any(np.asarray(bo) != 0)),
        bool(np.any(np.asarray(g0) != 1)),
        bool(np.any(np.asarray(b0) != 0)),
        bool(np.any(np.asarray(g1) != 1)),
        bool(np.any(np.asarray(b1) != 0)),
    )

    def w8(W):
        # W.T scaled by 16, fp8, laid out [p, e-chunk, e'] with e = c*128+p
        wt = (np.asarray(W, f32).T * 16.0).astype(E4)
        return np.ascontiguousarray(wt.reshape(2, P, D).transpose(1, 0, 2))

    WoT = np.asarray(Wo, f32).T
    wob = np.concatenate([WoT, np.ones((D, 1), f32)], axis=1).astype(BF)
    shared = {
        "wq8": w8(Wq),
        "wk8": w8(Wk),
        "wv8": w8(Wv),
        "wqtr": np.ascontiguousarray(np.asarray(Wq, f32).T),
        "wob": np.ascontiguousarray(wob.reshape(2, P, D + 1).transpose(1, 0, 2)),
        "wos": np.ascontiguousarray(-np.asarray(Wo, f32).sum(axis=1) / D),
    }
    opt = (
        ("bq2", 16.0 * np.asarray(bq, f32), flags[0]),
        ("bq", np.asarray(bq, f32), flags[0]),
        ("bk2", 16.0 * np.asarray(bk, f32), flags[1]),
        ("bv16", 16.0 * np.asarray(bv, f32), flags[2]),
        ("bo", np.asarray(bo, f32), flags[3]),
        ("g0", np.asarray(g0, f32), flags[4]),
        ("b0", np.asarray(b0, f32), flags[5]),
        ("g1", np.asarray(g1, f32), flags[6]),
        ("b1", np.asarray(b1, f32), flags[7]),
    )
    for nm, arr, used in opt:
        if used:
            shared[nm] = np.ascontiguousarray(arr)

    def t8(X):
        # X.T fp8 laid out [p, e-chunk, n] (weights carry the 16x scaling)
        xt = np.asarray(X, f32).T.astype(E4)
        return np.ascontiguousarray(xt.reshape(2, P, N).transpose(1, 0, 2))

    in_maps = []
    for b in range(B):
        m = dict(shared)
        m["qt8"] = t8(Q[b])
        m["kt8"] = t8(K[b])
        m["qtr"] = np.ascontiguousarray(Q[b].T)
        in_maps.append(m)
    return flags, in_maps


def run(trace=False, **inputs):
    flags, in_maps = _prep_inputs(**inputs)
    nc = _get_prog(flags)
    try:
        res = run_bass_kernel_spmd(nc, in_maps, list(range(B)), trace=trace)
    except ModuleNotFoundError:
        # NTFF profile hook unavailable in slim axon images
        res = run_bass_kernel_spmd(nc, in_maps, list(range(B)), trace=False)
    out = np.stack([res.results[b]["out"] for b in range(B)]).astype(np.float32)
    return out, res


def kernel(**inputs):
    out, _ = run(trace=False, **inputs)
    return out
